# revision 57
# baseline (speedup 1.0000x reference)
"""Trainium2 Bass kernel for Gemma4Audio chunked local attention.

Sharding: 8 cores = batch(4) x seq-half(2). Each core processes 3072
tokens of one batch (plus a 12-token left halo and 4-token right pad)
fully locally -- block-local attention never crosses the half-sequence
boundary mid-block, so no collectives are needed.

Optimized pipeline (617.8us -> 429.8us on the cost-model timeline):
- fp16 operands everywhere with fp32 PSUM accumulation; narrow fp16
  attention matmuls (112-wide scores, 96-wide outputs, fp16 PE
  transposes) instead of fp32r's >=256-wide requirement.
- three-iteration software pipelining: each loop iteration runs head
  h's projections/scores, head h-1's deferred softmax tail, and head
  h-3's transposes/output matmuls, so the scatter/mask/softcap/exp/
  normalize chain (DVE+ACT+GPSIMD) never stalls the tensor engine.
- softmax chain tail (lg-add/tanh/exp/reduce/recip/probs-normalize) is
  emitted one iteration late so its cross-engine waits never head-block
  the in-order DVE/ACT queues against next-iteration copies.
- post projection spread uniformly at one output chunk per iteration,
  with per-head aoT tiles so its dependencies stay per-head.
- weight DMAs head-major in two-head pairs (512B descriptors avoid the
  small-DMA 2x latency multiplier), x-input prefetched a region ahead.
"""

import math

import numpy as np

# ---- model constants (hardcoded per problem spec) ----
HID = 1024
H = 8
D = 128
CHUNK = 12
PAST = 12
CTX = 24  # context window per block
P = 25  # relative positions
SOFTCAP = 50.0
Q_SCALE = D ** (-0.5) / math.log(2)
K_SCALE = math.log(1.0 + math.e) / math.log(2)

B = 4
S = 6144
NCORES = 8

T = S // 2  # 3072 tokens per core
THALO = T + PAST + 4  # 3088 with left halo + right pad for 112-wide windows
TR = 384  # tokens per region
NREG = T // TR  # 8
TPB = 96  # queries per attention tile (8 blocks)
NTILE = TR // TPB  # 4
WIN = 112  # key window per tile (96 + 12 band + 4 pad, masked)
W = 108  # live key columns feeding the output matmul
KC = HID // 128  # 8 contraction chunks
NP25 = 26  # padded rel-position count for scatter (even)
DFREE = 112  # scatter destination free size per tile
MASKVAL = -30000.0  # fits fp16; tanh saturates -> exp(-50) ~ 0

_CACHE = {}


def _build_tables():
    """Host-precomputed scatter index tables and band mask (batched over
    the NTILE tiles of a head-region: targets offset by g*DFREE)."""
    idx = np.full((128, NTILE * 2 * NP25), -1, dtype=np.int16)
    mask = np.full((128, NTILE * DFREE), MASKVAL, dtype=np.float16)
    for a in range(TPB):
        i, c = divmod(a, CHUNK)
        for g in range(NTILE):
            mask[a, g * DFREE + 12 * i : g * DFREE + 12 * i + CTX] = 0.0
            for p in range(P):
                # term A: own-row rel score at window col a+p (ctx col c+p)
                if c + p < CTX:
                    idx[a, g * 2 * NP25 + p] = g * DFREE + a + p
                # term B (rel_shift row leak): prev query's rel score
                if p >= P - c:
                    idx[a, g * 2 * NP25 + NP25 + p] = g * DFREE + a + p - P
    return idx, mask


def _build_bass():
    import concourse.bass as bass
    import concourse.bacc as bacc
    import concourse.mybir as mybir
    import concourse.tile as tile
    from concourse.masks import make_identity

    dt = mybir.dt
    f32 = dt.float32
    f16 = dt.float16
    AF = mybir.ActivationFunctionType
    ADD = mybir.AluOpType.add
    AXX = mybir.AxisListType.X

    nc = bacc.Bacc(None, target_bir_lowering=False)

    xT = nc.declare_dram_parameter("xT", [HID, THALO], f16, isOutput=False)
    wqT = nc.declare_dram_parameter("wqT", [HID, HID], f16, isOutput=False)
    wkT = nc.declare_dram_parameter("wkT", [HID, HID], f16, isOutput=False)
    wvT = nc.declare_dram_parameter("wvT", [HID, HID], f16, isOutput=False)
    wpT = nc.declare_dram_parameter("wpT", [HID, HID], f16, isOutput=False)
    wrelT = nc.declare_dram_parameter("wrelT", [HID, HID], f16, isOutput=False)
    pembT = nc.declare_dram_parameter("pembT", [HID, 32], f16, isOutput=False)
    idxtab = nc.declare_dram_parameter("idxtab", [128, NTILE * 2 * NP25], dt.int16, isOutput=False)
    masktab = nc.declare_dram_parameter("masktab", [128, NTILE * DFREE], f16, isOutput=False)
    outT = nc.declare_dram_parameter("outT", [HID, T], f32, isOutput=True)

    with tile.TileContext(nc) as tc:
        with (
            tc.tile_pool(name="consts", bufs=1) as cpool,
            tc.tile_pool(name="pj", bufs=3, space="PSUM") as pjpool,
            tc.tile_pool(name="psS", bufs=1, space="PSUM") as pspoolS,
            tc.tile_pool(name="psB", bufs=1, space="PSUM") as pspoolB,
            tc.tile_pool(name="psT", bufs=2, space="PSUM") as pspoolT,
            tc.tile_pool(name="psO", bufs=1, space="PSUM") as pspoolO,
        ):
            idx_sb = cpool.tile([128, NTILE * 2 * NP25], dt.int16, tag="idx")
            nc.sync.dma_start(out=idx_sb[:], in_=idxtab[:, :])
            mask_sb = cpool.tile([128, NTILE * DFREE], f16, tag="mask")
            nc.sync.dma_start(out=mask_sb[:], in_=masktab[:, :])
            ident = cpool.tile([128, 128], f16, tag="ident")
            make_identity(nc, ident[:])
            relk_sb = cpool.tile([128, H, 32], f16, tag="relk")

            with (
                tc.tile_pool(name="weights", bufs=1) as wpool,
                tc.tile_pool(name="wrelp", bufs=2) as wrelpool,
                tc.tile_pool(name="xin", bufs=2) as xpool,
                tc.tile_pool(name="strips", bufs=2) as spool,
                tc.tile_pool(name="ao", bufs=2) as aopool,
                tc.tile_pool(name="attn", bufs=2) as apool,
            ):
                w_sb = {}
                wviews = {}
                for name, drh in (("wq", wqT), ("wk", wkT), ("wv", wvT), ("wp", wpT)):
                    w_sb[name] = wpool.tile([128, KC, HID], f16, tag=name, name=name)
                    wviews[name] = drh.rearrange("(kc p) o -> p kc o", p=128)
                pemb_sb = wrelpool.tile([128, KC, 32], f16, tag="pemb")
                nc.sync.dma_start(
                    out=pemb_sb[:], in_=pembT.rearrange("(kc p) o -> p kc o", p=128)
                )
                _main(nc, tc, mybir, AF, ADD, AXX, w_sb, wviews, xT, wrelT, outT,
                      idx_sb, mask_sb, ident, relk_sb, pemb_sb, wrelpool, xpool,
                      spool, aopool, apool, pjpool, pspoolS, pspoolB, pspoolT,
                      pspoolO, f32, f16)
    nc.compile()
    return nc


def _main(nc, tc, mybir, AF, ADD, AXX, w_sb, wviews, xT, wrelT, outT, idx_sb,
          mask_sb, ident, relk_sb, pemb_sb, wrelpool, xpool, spool, aopool,
          apool, pjpool, pspoolS, pspoolB, pspoolT, pspoolO, f32, f16):
    NIT = NREG * H  # 64 head-region iterations
    STAG = 3  # software-pipeline stagger (covers the softmax chain latency)

    # per-iteration live state, keyed it -> dict
    st = {}
    xr_by_reg = {}
    aoT_by_reg = {}
    wrel_by_h = {}
    marks = _CACHE.setdefault("stage_marks", [])

    def mark(label):
        marks.append((nc.next_id(), label))

    def prefetch_xr(r):
        xr = xpool.tile([128, KC, TR + 16], f16, tag="xr", name="xr")
        xsrc = xT[:, r * TR : r * TR + TR + 16].rearrange(
            "(kc p) n -> p kc n", p=128
        )
        for kc in range(KC):
            nc.sync.dma_start(out=xr[:, kc, :], in_=xsrc[:, kc, :])
        xr_by_reg[r] = xr

    wrel_all = wrelpool.tile([128, KC, H * 128], f16, tag="wrel", name="wrel_all")

    def prefetch_wrel(h0):
        # two heads per transfer: 512-byte descriptors avoid the small-DMA
        # 2x latency multiplier
        nc.sync.dma_start(
            out=wrel_all[:, :, h0 * 128 : (h0 + 2) * 128],
            in_=wrelT[:, h0 * 128 : (h0 + 2) * 128].rearrange(
                "(kc p) o -> p kc o", p=128
            ),
        )

    def stage_relk(h):
        """rel_k = Wrel_h @ pos_emb.T, inlined into region-0 iteration h."""
        ps = pjpool.tile([128, 32], f32, tag="pj")
        for kc in range(KC):
            nc.tensor.matmul(
                ps[:], lhsT=wrel_all[:, kc, h * 128 : (h + 1) * 128],
                rhs=pemb_sb[:, kc, :],
                start=(kc == 0), stop=(kc == KC - 1),
            )
        nc.vector.tensor_copy(relk_sb[:, h, :], ps[:])

    def stage_front(it):
        """Projections for iteration `it` = (r, h)."""
        r, h = divmod(it, H)
        xr = xr_by_reg[r]
        hs = slice(h * 128, (h + 1) * 128)

        # --- projections (fp16 operands, fp32 PSUM accumulate) ---
        qps = pjpool.tile([128, TR + 2], f32, tag="pj")
        kps = pjpool.tile([128, TR + 16], f32, tag="pj")
        vps = pjpool.tile([128, TR + 12], f32, tag="pj")
        for kc in range(KC):
            st_, sp_ = kc == 0, kc == KC - 1
            nc.tensor.matmul(
                qps[:], lhsT=w_sb["wq"][:, kc, hs], rhs=xr[:, kc, 10 : TR + 12],
                start=st_, stop=sp_,
            )
        for kc in range(KC):
            st_, sp_ = kc == 0, kc == KC - 1
            nc.tensor.matmul(
                kps[:], lhsT=w_sb["wk"][:, kc, hs], rhs=xr[:, kc, :],
                start=st_, stop=sp_,
            )
        for kc in range(KC):
            st_, sp_ = kc == 0, kc == KC - 1
            nc.tensor.matmul(
                vps[:], lhsT=w_sb["wv"][:, kc, hs], rhs=xr[:, kc, 0 : TR + 12],
                start=st_, stop=sp_,
            )
        q_sb = spool.tile([128, TR + 2], f16, tag="q")
        k_sb = spool.tile([128, TR + 16], f16, tag="k")
        v_sb = spool.tile([128, TR + 12], f16, tag="v", bufs=STAG + 1)
        nc.vector.tensor_copy(q_sb[:], qps[:])
        nc.vector.tensor_copy(k_sb[:], kps[:])
        nc.scalar.copy(v_sb[:], vps[:])
        st[it] = dict(q=q_sb, k=k_sb, v=v_sb)

    def stage_scores(it):
        """Content + rel-position score matmuls for iteration `it`."""
        r, h = divmod(it, H)
        s = st[it]
        q_sb, k_sb = s["q"], s["k"]
        sall = pspoolS.tile([TPB, NTILE, WIN], f32, tag="sall")
        bdall = pspoolB.tile([TPB, NTILE, 2 * NP25], f32, tag="bd")
        for g in range(NTILE):
            b0 = TPB * g
            qmain = q_sb[:, b0 + 2 : b0 + 2 + TPB]
            qprev = q_sb[:, b0 + 1 : b0 + 1 + TPB]
            nc.tensor.matmul(
                sall[:, g, :], lhsT=qmain, rhs=k_sb[:, b0 : b0 + WIN],
                start=True, stop=True,
            )
            nc.tensor.matmul(
                bdall[:, g, 0:NP25], lhsT=qmain, rhs=relk_sb[:, h, 0:NP25],
                start=True, stop=True,
            )
            nc.tensor.matmul(
                bdall[:, g, NP25 : 2 * NP25], lhsT=qprev, rhs=relk_sb[:, h, 0:NP25],
                start=True, stop=True,
            )
        s["sall"], s["bdall"] = sall, bdall

    def stage_softmax(it):
        """Scatter + mask + softcap + exp + normalize -> fp16 probs.

        Chain engine plan: DVE(bd copy) -> Pool(scatter, mask, lg-add) ->
        ACT(tanh, exp) -> Pool(reduce, divide). Keeps the DVE queue free of
        mid-chain waits so next-iteration copies aren't head-blocked.
        """
        s = st[it]
        data = apool.tile([TPB, NTILE, 2 * NP25], f16, tag="data")
        nc.vector.tensor_copy(data[:], s["bdall"][:])
        dst = apool.tile([TPB, NTILE * DFREE], f16, tag="dst")
        nc.gpsimd.local_scatter(
            dst[:], data[:], idx_sb[0:TPB, :],
            channels=TPB, num_elems=NTILE * DFREE, num_idxs=NTILE * 2 * NP25,
        )
        nc.gpsimd.tensor_tensor(
            out=dst[:], in0=dst[:], in1=mask_sb[0:TPB, :], op=ADD,
        )
        s["dst"] = dst

    def stage_normalize(it):
        """Deferred chain tail, emitted at the start of the NEXT iteration:
        by then scatter+mask are (nearly) done, so these ops never
        head-block their queues against upstream engines for long."""
        s = st[it]
        lg = apool.tile([TPB, NTILE, DFREE], f32, tag="lg")
        nc.vector.tensor_tensor(
            out=lg[:], in0=s["sall"][:],
            in1=s["dst"].rearrange("p (g w) -> p g w", g=NTILE), op=ADD,
        )
        nc.scalar.activation(out=lg[:], in_=lg[:], func=AF.Tanh, scale=1.0 / SOFTCAP)
        nc.scalar.activation(out=lg[:], in_=lg[:], func=AF.Exp, scale=SOFTCAP)
        rsum = apool.tile([TPB, NTILE], f32, tag="rsum")
        nc.vector.tensor_reduce(out=rsum[:], in_=lg[:], axis=AXX, op=ADD)
        nc.vector.reciprocal(rsum[:], rsum[:])
        pr = apool.tile([TPB, NTILE, W], f16, tag="pr", bufs=STAG + 1)
        for g in range(NTILE):
            nc.gpsimd.tensor_scalar_mul(
                out=pr[:, g, :], in0=lg[:, g, 0:W], scalar1=rsum[:, g : g + 1],
            )
        s["pr"] = pr

    def stage_transpose(it):
        """PE transposes of probs + V for iteration `it` (ready long ago)."""
        s = st[it]
        pr, v_sb = s["pr"], s["v"]
        atvt = pspoolT.tile([W, NTILE, TPB + 128], f16, tag="atvt")
        for g in range(NTILE):
            b0 = TPB * g
            nc.tensor.transpose(
                atvt[:, g, 0:TPB], pr[:, g, :], ident[0:TPB, 0:TPB]
            )
            nc.tensor.transpose(
                atvt[:, g, TPB : TPB + 128], v_sb[:, b0 : b0 + W], ident[:, :]
            )
        s["atvt"] = atvt

    def stage_copies(it):
        """PSUM->SBUF copies of the transposed tiles."""
        s = st[it]
        at_sb = apool.tile([W, NTILE, TPB], f16, tag="at")
        vt_sb = apool.tile([W, NTILE, 128], f16, tag="vt")
        nc.vector.tensor_copy(at_sb[:], s["atvt"][:, :, 0:TPB])
        nc.vector.tensor_copy(vt_sb[:], s["atvt"][:, :, TPB : TPB + 128])
        s["at"], s["vt"] = at_sb, vt_sb

    def stage_out(it):
        """Attention-output matmuls + aoT copy (per-head tile so the post
        projection's dependencies stay per-head)."""
        r, h = divmod(it, H)
        s = st[it]
        aops = pspoolO.tile([128, NTILE, TPB], f32, tag="aops")
        for g in range(NTILE):
            nc.tensor.matmul(
                aops[:, g, :], lhsT=s["vt"][:, g, :], rhs=s["at"][:, g, :],
                start=True, stop=True,
            )
        if h == 0:
            aoT_by_reg[r] = [
                aopool.tile([128, TR], f16, tag=f"aoT{hh}", name=f"aoT{hh}")
                for hh in range(H)
            ]
        nc.vector.tensor_copy(aoT_by_reg[r][h][:], aops[:])
        del st[it]

    def emit_post(r, ocs, alternate=False):
        """Post projection for region r, given output-channel chunks."""
        aoT = aoT_by_reg[r]
        for oc in ocs:
            pps = pjpool.tile([128, TR], f32, tag="pj")
            for h in range(H):
                nc.tensor.matmul(
                    pps[:],
                    lhsT=w_sb["wp"][:, h, oc * 128 : (oc + 1) * 128],
                    rhs=aoT[h][:],
                    start=(h == 0), stop=(h == H - 1),
                )
            po = apool.tile([128, TR], f32, tag="po", bufs=4)
            if alternate and oc % 2 == 1:
                nc.vector.tensor_copy(po[:], pps[:])
            else:
                nc.scalar.copy(po[:], pps[:])
            nc.sync.dma_start(
                out=outT[oc * 128 : (oc + 1) * 128, r * TR : (r + 1) * TR],
                in_=po[:],
            )

    # ---- software-pipelined main loop (STAG-iteration stagger) ----
    # DMA issue order matters (single in-order queue): first-iteration
    # input, then per-head weight slices (wq/wk/wv then wrel) head-major
    # so head h's slices stay ahead of iteration h, wp (first needed at
    # region-0 post) last.
    nc.sync.dma_start(
        out=w_sb["wq"][:, :, 0:256], in_=wviews["wq"][:, :, 0:256]
    )
    prefetch_xr(0)
    for h0 in range(0, H, 2):
        hs = slice(h0 * 128, (h0 + 2) * 128)
        for name in ("wq", "wk", "wv"):
            if name == "wq" and h0 == 0:
                continue
            nc.sync.dma_start(out=w_sb[name][:, :, hs], in_=wviews[name][:, :, hs])
        prefetch_wrel(h0)
    for h0 in range(0, H, 2):
        hs = slice(h0 * 128, (h0 + 2) * 128)
        nc.sync.dma_start(out=w_sb["wp"][:, :, hs], in_=wviews["wp"][:, :, hs])
    def post_for(it):
        """Uniform 1-oc-per-iteration post schedule: iteration (r, h) does
        oc h-2 of region r-1 for h in 2..7, and oc 6+h of region r-2 for
        h in 0..1. Region 7's chunks are flushed after the loop."""
        r, h = divmod(it, H)
        if h >= 2 and r >= 1:
            return (r - 1, h - 2)
        if h <= 1 and r >= 2:
            return (r - 2, 6 + h)
        return None

    for it in range(NIT + STAG):
        fin = it - STAG
        if fin >= 0:
            mark(f"it{it}:transpose")
            stage_transpose(fin)
            mark(f"it{it}:copies")
            stage_copies(fin)
        if it < NIT:
            mark(f"it{it}:front")
            stage_front(it)
        if fin >= 0:
            mark(f"it{it}:out")
            stage_out(fin)
        if 0 <= it - 1 < NIT:
            mark(f"it{it}:normalize")
            stage_normalize(it - 1)
        if it < NIT:
            r, h = divmod(it, H)
            pf = post_for(it)
            if pf is not None:
                mark(f"it{it}:post")
                emit_post(pf[0], [pf[1]])
            if it < H:
                mark(f"it{it}:relk")
                stage_relk(it)
            mark(f"it{it}:scores")
            stage_scores(it)
            mark(f"it{it}:softmax")
            stage_softmax(it)
            if h == 5 and r + 1 < NREG:
                mark(f"it{it}:xrpf")
                prefetch_xr(r + 1)
        elif it - NIT < 2:
            # drain: region 6's last two chunks land here
            mark(f"it{it}:post")
            emit_post(NREG - 2, [6 + (it - NIT)])
    mark("final_post")
    emit_post(NREG - 1, list(range(KC)), alternate=True)
    mark("end")


def _get_nc():
    if "nc" not in _CACHE:
        _CACHE["nc"] = _build_bass()
    return _CACHE["nc"]


def _prepare_in_maps(hidden_states, position_embeddings, Wq, Wk, Wv, Wpost, Wrel,
                     per_dim_scale):
    f16 = np.float16
    hs = np.asarray(hidden_states, np.float32)
    pe = np.asarray(position_embeddings, np.float32)
    qscale = (Q_SCALE * np.log1p(np.exp(np.asarray(per_dim_scale, np.float64)))).astype(
        np.float64
    )
    qs_tiled = np.tile(qscale, H)  # per output channel o: scale[o % 128]
    wqT = np.ascontiguousarray(
        (np.asarray(Wq, np.float64) * qs_tiled[:, None]).T.astype(f16)
    )
    wkT = np.ascontiguousarray((np.asarray(Wk, np.float64) * K_SCALE).T.astype(f16))
    wvT = np.ascontiguousarray(np.asarray(Wv, np.float32).T.astype(f16))
    wpT = np.ascontiguousarray(np.asarray(Wpost, np.float32).T.astype(f16))
    wrelT = np.ascontiguousarray(np.asarray(Wrel, np.float32).T.astype(f16))
    pembT = np.zeros((HID, 32), f16)
    pembT[:, :P] = pe.T.astype(f16)
    idx, mask = _build_tables()

    shared = dict(wqT=wqT, wkT=wkT, wvT=wvT, wpT=wpT, wrelT=wrelT, pembT=pembT,
                  idxtab=idx, masktab=mask)
    in_maps = []
    for core in range(NCORES):
        b, half = divmod(core, 2)
        lo = half * T
        slab = np.zeros((THALO, HID), np.float32)
        src_lo = max(lo - PAST, 0)
        src_hi = min(lo + T + 4, S)
        off = src_lo - (lo - PAST)
        slab[off : off + (src_hi - src_lo), :] = hs[b, src_lo:src_hi, :]
        xT = np.ascontiguousarray(slab.T.astype(f16))
        in_maps.append(dict(xT=xT, **shared))
    return in_maps


def _assemble(results):
    out = np.empty((B, S, HID), np.float32)
    for core in range(NCORES):
        b, half = divmod(core, 2)
        out[b, half * T : (half + 1) * T, :] = results[core]["outT"].T
    return out


def kernel(**inputs) -> np.ndarray:
    from concourse.bass_utils import run_bass_kernel_spmd

    nc = _get_nc()
    in_maps = _prepare_in_maps(**inputs)
    res = run_bass_kernel_spmd(nc, in_maps, list(range(NCORES)))
    return _assemble(res.results)
